# revision 24
# baseline (speedup 1.0000x reference)
"""Trainium2 Bass kernel for nn_AEModel (VQ autoencoder), 8-core data-parallel.

Architecture: Dense(D->H) -> BN(train) -> ReLU -> Dense(H->dm) -> VQ nearest
codebook (K codes) -> Dense(dm->H) -> BN -> ReLU -> Dense(H->D) -> ReLU.
Returns (pred [B,D], assignment=-dist [B,K], feat [B,dm]).

Sharding: data-parallel over batch (B/8 rows per core); weights replicated;
BN statistics all-reduced (2 x 16KB collectives).

Precision: encoder + VQ distance use fp16 hi/lo split matmuls (3 matmuls per
fp32 matmul, ~22-bit effective mantissa) so the argmin matches the CPU fp32
reference exactly; decoder uses plain fp16 (tolerance is ample post-VQ).
Quantization is one-hot: onehot = (assign == rowmax) compared against values
from the same tensor, then quant = C^T @ onehot^T via PE transposes.
"""

import sys

try:
    import concourse.bass as bass  # noqa: F401
except ImportError:
    sys.path.insert(0, "/opt/trn_rl_repo")

import numpy as np

import concourse.bass as bass  # noqa: F811
import concourse.mybir as mybir
import concourse.tile as tile
from concourse import bacc
from concourse.bass_utils import run_bass_kernel_spmd
from concourse.masks import make_identity

F32 = mybir.dt.float32
F16 = mybir.dt.float16

BN_EPS = 1e-3


class Cfg:
    def __init__(self, B=4096, D=4096, H=2048, dm=128, K=4096, n_cores=8):
        self.B, self.D, self.H, self.dm, self.K, self.n_cores = B, D, H, dm, K, n_cores
        self.Bs = B // n_cores          # batch rows per core
        self.BT = self.Bs // 128        # batch row-tiles per core
        self.DC = D // 128              # input-dim chunks
        self.HC = H // 128              # hidden chunks
        self.KC = K // 128              # code chunks (partition-sized)
        self.KN = K // 512              # code chunks (free-dim-sized)
        self.DN = D // 512              # output-dim chunks
        assert self.Bs % 128 == 0 and D % 128 == 0 and H % 128 == 0
        assert dm == 128 and K % 512 == 0 and D % 512 == 0


FULL_CFG = Cfg()


def build_graph(cfg: Cfg, has_bd2: bool = False):
    """Build the SPMD graph (same graph runs on all cores)."""
    nc = bacc.Bacc(
        "TRN2",
        target_bir_lowering=False,
        debug=False,
        num_devices=cfg.n_cores,
    )
    Bs, DC, HC, KC, DN = cfg.Bs, cfg.DC, cfg.HC, cfg.KC, cfg.DN
    K, dm = cfg.K, cfg.dm

    di = lambda n, s, d=F16: nc.dram_tensor(n, s, d, kind="ExternalInput").ap()
    t = {}
    t["xt_hi"] = di("xt_hi", [DC, 128, Bs])      # x^T shard, split, dc-major
    t["xt_lo"] = di("xt_lo", [DC, 128, Bs])
    t["we1_hi"] = di("we1_hi", [DC, 128, HC, 128])
    t["we1_lo"] = di("we1_lo", [DC, 128, HC, 128])
    t["we2_hi"] = di("we2_hi", [128, HC, dm])    # [h-part, hc, dm]
    t["we2_lo"] = di("we2_lo", [128, HC, dm])
    t["ct2_hi"] = di("ct2_hi", [dm, K])          # 2*C^T, split
    t["ct2_lo"] = di("ct2_lo", [dm, K])
    t["csqn"] = di("csqn", [2, K])               # rows: hi, lo of (-|c|^2)
    t["c_rows"] = di("c_rows", [K, dm])          # C fp16 row table (gather)
    t["wd1"] = di("wd1", [dm, cfg.H])
    t["wd2"] = di("wd2", [DN, 128, HC, 512])     # [dn, h-part, hc, d-free]
    t["bn1v"] = di("bn1v", [128, 2, HC], F32)    # [:,0]=ge striped, [:,1]=bbe
    t["bn2v"] = di("bn2v", [128, 2, HC], F32)
    t["be2c"] = di("be2c", [dm, 1], F32)
    if has_bd2:
        t["bd2r"] = di("bd2r", [2, cfg.D])

    do = lambda n, s: nc.dram_tensor(n, s, F32, kind="ExternalOutput").ap()
    t["pred_o"] = do("pred", [Bs, cfg.D])
    t["assign_o"] = do("assign", [Bs, K])
    t["feat_o"] = do("feat", [Bs, dm])

    with tile.TileContext(nc) as tc:
        _body(tc, nc, cfg, t, has_bd2)
    nc.compile()
    return nc


def _body(tc, nc, cfg, t, has_bd2):
    Bs, BT, DC, HC, KC, KN, DN = (
        cfg.Bs, cfg.BT, cfg.DC, cfg.HC, cfg.KC, cfg.KN, cfg.DN)
    K, dm, B, H = cfg.K, cfg.dm, cfg.B, cfg.H
    groups = [list(range(cfg.n_cores))]
    AF = mybir.ActivationFunctionType
    OP = mybir.AluOpType
    AX = mybir.AxisListType

    from contextlib import ExitStack
    ctx = ExitStack()
    with ctx:
        P_res = ctx.enter_context(tc.tile_pool(name="res", bufs=1))
        P_dram = ctx.enter_context(tc.tile_pool(name="drp", bufs=1, space="DRAM"))
        P_tmp = ctx.enter_context(tc.tile_pool(name="tmp", bufs=3))

        def R(shape, dtype, name):       # resident tile, own slot
            return P_res.tile(shape, dtype, name=name, tag=name)

        # ---------- resident loads ----------
        ident16 = R([128, 128], F16, "ident16")
        make_identity(nc, ident16)
        ident32 = R([128, 128], F32, "ident32")
        make_identity(nc, ident32)

        def load(name, shape, dtype=F16):
            sb = R(shape, dtype, name + "_sb")
            nc.sync.dma_start(sb, t[name])
            return sb


        stat1 = R([128, HC, 2], F32, "stat1")    # [:, hc, 0]=sum, [:, hc, 1]=sumsq
        tot1 = R([128, HC, 2], F32, "tot1")
        stat2 = R([128, 2 * HC], F32, "stat2")
        fsq = R([128, BT], F32, "fsq")
        fsqn = R([128, BT], F32, "fsqn")
        bn1_sb = load("bn1v", [128, 2, HC], F32)

        # ---------- Phase A: h^T = We1^T @ x^T (fp16 split x3) ----------
        # dc-outer / hc-inner over groups of GH psum banks; x^T and We1 both
        # streamed (x^T is re-read once per group). BN1 statistics are
        # all-reduced per group so the collective overlaps the next group's
        # matmuls.
        ctx_abc = ExitStack()
        P_h = ctx_abc.enter_context(tc.tile_pool(name="hsb", bufs=1))
        h_sb = [P_h.tile([128, Bs], F32, name=f"h{hc}", tag=f"h{hc}")
                for hc in range(HC)]
        GH = min(8, HC)
        NGRP = HC // GH
        with tc.tile_pool(name="xts", bufs=3) as P_xts, \
             tc.tile_pool(name="w1", bufs=3) as P_w1, \
             tc.tile_pool(name="psA", bufs=1, space="PSUM") as P_psA:
            for g in range(NGRP):
                pss = [P_psA.tile([128, Bs], F32, name=f"hps{j}", tag=f"hps{j}")
                       for j in range(GH)]
                for dc in range(DC):
                    xh = P_xts.tile([128, Bs], F16, name="xsh", tag="xsh")
                    nc.sync.dma_start(xh, t["xt_hi"][dc])
                    xl = P_xts.tile([128, Bs], F16, name="xsl", tag="xsl")
                    nc.sync.dma_start(xl, t["xt_lo"][dc])
                    gs = slice(g * GH, (g + 1) * GH)
                    wh = P_w1.tile([128, GH, 128], F16, name="w1h", tag="w1h")
                    nc.sync.dma_start(wh, t["we1_hi"][dc, :, gs, :])
                    wl = P_w1.tile([128, GH, 128], F16, name="w1l", tag="w1l")
                    nc.sync.dma_start(wl, t["we1_lo"][dc, :, gs, :])
                    for j in range(GH):
                        ps = pss[j]
                        nc.tensor.matmul(ps, wh[:, j], xh,
                                         start=(dc == 0), stop=False)
                        nc.tensor.matmul(ps, wh[:, j], xl,
                                         start=False, stop=False)
                        nc.tensor.matmul(ps, wl[:, j], xh,
                                         start=False, stop=(dc == DC - 1))
                for j in range(GH):
                    hc = g * GH + j
                    nc.scalar.activation(h_sb[hc], pss[j], AF.Identity,
                                         accum_out=stat1[:, hc, 0:1])
                    sq = P_tmp.tile([128, Bs], F32, name="sq", tag="sq")
                    nc.scalar.activation(sq, h_sb[hc], AF.Square,
                                         accum_out=stat1[:, hc, 1:2])
                # group all-reduce (overlaps the next group's matmuls)
                gs = slice(g * GH, (g + 1) * GH)
                sin = P_dram.tile([128, GH, 2], F32, name=f"b1i{g}", tag=f"b1i{g}")
                sout = P_dram.tile([128, GH, 2], F32, name=f"b1o{g}", tag=f"b1o{g}")
                nc.sync.dma_start(sin, stat1[:, gs, :])
                nc.gpsimd.collective_compute(
                    "AllReduce", OP.add, replica_groups=groups,
                    ins=[sin.opt()], outs=[sout.opt()])
                nc.sync.dma_start(tot1[:, gs, :], sout)

        if True:
            # ---------- Phase B/C: BN apply + feat^T, per group ----------
            we2_hi_sb = load("we2_hi", [128, HC, dm])
            we2_lo_sb = load("we2_lo", [128, HC, dm])
            be2_sb = load("be2c", [dm, 1], F32)
            scale1 = R([128, HC], F32, "scale1")
            bias1 = R([128, HC], F32, "bias1")
            P_psum = ctx_abc.enter_context(
                tc.tile_pool(name="psBC", bufs=2, space="PSUM"))
            hn_hi = [P_h.tile([128, Bs], F16, name=f"hnh{hc}", tag=f"hnh{hc}")
                     for hc in range(HC)]
            hn_lo = [P_h.tile([128, Bs], F16, name=f"hnl{hc}", tag=f"hnl{hc}")
                     for hc in range(HC)]
            fps = P_psum.tile([dm, Bs], F32, name="fps", tag="fps", bufs=1)
            for g in range(NGRP):
                gs = slice(g * GH, (g + 1) * GH)
                _bn_math(nc, P_tmp, tot1[:, gs, 0], tot1[:, gs, 1],
                         bn1_sb[:, 0, gs], bn1_sb[:, 1, gs],
                         scale1[:, gs], bias1[:, gs], f"bn1g{g}", B, GH)
                for j in range(GH):
                    hc = g * GH + j
                    hn = P_tmp.tile([128, Bs], F32, name="hn", tag="hn")
                    nc.scalar.activation(hn, h_sb[hc], AF.Relu,
                                         bias=bias1[:, hc:hc + 1],
                                         scale=scale1[:, hc:hc + 1])
                    nc.vector.tensor_copy(hn_hi[hc], hn)
                    nc.vector.tensor_tensor(hn_lo[hc], hn, hn_hi[hc], OP.subtract)
                    nc.tensor.matmul(fps, we2_hi_sb[:, hc], hn_hi[hc],
                                     start=(hc == 0), stop=False)
                    nc.tensor.matmul(fps, we2_hi_sb[:, hc], hn_lo[hc],
                                     start=False, stop=False)
                    nc.tensor.matmul(fps, we2_lo_sb[:, hc], hn_hi[hc],
                                     start=False, stop=(hc == HC - 1))

            featT = R([dm, Bs], F32, "featT")
            nc.scalar.activation(featT, fps, AF.Identity, bias=be2_sb[:, 0:1])
            fT_hi = R([dm, Bs], F16, "fT_hi")
            nc.vector.tensor_copy(fT_hi, featT)
            fT_lo = R([dm, Bs], F16, "fT_lo")
            nc.vector.tensor_tensor(fT_lo, featT, fT_hi, OP.subtract)

            # feat output ([Bs, dm]) + per-row |f|^2 via PE transpose
            for bt in range(BT):
                tp = P_psum.tile([128, 128], F32, name="tps32", tag="tps32")
                nc.tensor.transpose(tp, featT[:, bt * 128:(bt + 1) * 128], ident32)
                fb = P_tmp.tile([128, dm], F32, name="fbk", tag="fbk")
                nc.scalar.activation(fb, tp, AF.Identity)
                nc.sync.dma_start(t["feat_o"][bt * 128:(bt + 1) * 128, :], fb)
                sqf = P_tmp.tile([128, dm], F32, name="sqf", tag="sqf")
                nc.scalar.activation(sqf, fb, AF.Square,
                                     accum_out=fsq[:, bt:bt + 1])
            nc.vector.tensor_scalar(fsqn, fsq, -1.0, None, OP.mult)

        ctx_abc.close()

        # ---------- Phase D+E: assignment/-dist, onehot, quant^T ----------
        ct2_hi_sb = load("ct2_hi", [dm, K])
        ct2_lo_sb = load("ct2_lo", [dm, K])
        # -|c|^2 as rows 0..1 of a zero-padded [128, K] tile so the csq add is
        # a full-K matmul (K=2 matmuls measured 2.2x slower than K=128).
        csqn_pad = R([128, K], F16, "csqn_pad")
        nc.vector.memset(csqn_pad, 0.0)
        nc.sync.dma_start(csqn_pad[0:2, :], t["csqn"])
        ones_pad = R([128, 128], F16, "ones_pad")
        nc.vector.memset(ones_pad, 0.0)
        nc.vector.memset(ones_pad[0:2, :], 1.0)

        quant16 = R([dm, Bs], F16, "quant16")
        wd1_sb = load("wd1", [dm, H])
        bn2_sb = load("bn2v", [128, 2, HC], F32)
        stat2p = R([128, 2, HC, BT], F32, "stat2p")
        ctx_fg = ExitStack()
        P_fg = ctx_fg.enter_context(tc.tile_pool(name="fg", bufs=1))
        d1_sb = [P_fg.tile([128, Bs], F32, name=f"d1{hc}", tag=f"d1{hc}")
                 for hc in range(HC)]
        with tc.tile_pool(name="asn", bufs=2) as P_asn, \
             tc.tile_pool(name="psD", bufs=2, space="PSUM") as P_psum:
            for bt in range(BT):
                asn = P_asn.tile([128, K], F32, name="asn", tag="asn")
                fh = fT_hi[:, bt * 128:(bt + 1) * 128]
                fl = fT_lo[:, bt * 128:(bt + 1) * 128]
                for kn in range(KN):
                    ks = slice(kn * 512, (kn + 1) * 512)
                    dp = P_psum.tile([128, 512], F32, name="dps", tag="dps")
                    nc.tensor.matmul(dp, fh, ct2_hi_sb[:, ks], start=True, stop=False)
                    nc.tensor.matmul(dp, fh, ct2_lo_sb[:, ks], start=False, stop=False)
                    nc.tensor.matmul(dp, fl, ct2_hi_sb[:, ks], start=False, stop=False)
                    nc.tensor.matmul(dp, ones_pad, csqn_pad[:, ks],
                                     start=False, stop=True)
                    nc.scalar.activation(asn[:, ks], dp, AF.Identity,
                                         bias=fsqn[:, bt:bt + 1])
                nc.sync.dma_start(t["assign_o"][bt * 128:(bt + 1) * 128, :], asn)
                # hardware argmax (top-8 values + indices), then gather the
                # winning codebook rows and transpose into quant^T columns
                mx8 = P_tmp.tile([128, 8], F32, name="mx8", tag="mx8")
                nc.vector.max(out=mx8, in_=asn)
                idx8 = P_tmp.tile([128, 8], mybir.dt.uint32, name="idx8", tag="idx8")
                nc.vector.max_index(idx8, mx8, asn)
                qbk = P_tmp.tile([128, dm], F16, name="qbk", tag="qbk")
                nc.gpsimd.indirect_dma_start(
                    out=qbk, out_offset=None, in_=t["c_rows"],
                    in_offset=bass.IndirectOffsetOnAxis(ap=idx8[:, 0:1], axis=0))
                tq = P_psum.tile([128, dm], F16, name="tq", tag="tq")
                nc.tensor.transpose(tq, qbk, ident16)
                bs_ = slice(bt * 128, (bt + 1) * 128)
                nc.scalar.activation(quant16[:, bs_], tq, AF.Identity)
                # d1^T column block for this bt + its BN2 stat contributions
                for hcg in range(HC // 4):
                    d1p = P_psum.tile([128, 4, 128], F32, name="d1ps", tag="d1ps")
                    for j in range(4):
                        hc = hcg * 4 + j
                        nc.tensor.matmul(
                            d1p[:, j], wd1_sb[:, hc * 128:(hc + 1) * 128],
                            quant16[:, bs_], start=True, stop=True)
                    for j in range(4):
                        hc = hcg * 4 + j
                        nc.scalar.activation(
                            d1_sb[hc][:, bs_], d1p[:, j], AF.Identity,
                            accum_out=stat2p[:, 0, hc, bt:bt + 1])
                        sq = P_tmp.tile([128, 128], F32, name="sqd", tag="sqd")
                        nc.scalar.activation(
                            sq, d1_sb[hc][:, bs_], AF.Square,
                            accum_out=stat2p[:, 1, hc, bt:bt + 1])

        # ---------- Phase F: BN2 all-reduce + apply ----------
        nc.vector.tensor_reduce(stat2, stat2p, axis=AX.X, op=OP.add)
        scale2, bias2 = _bn_allreduce(
            tc, nc, P_res, P_dram, P_tmp, stat2, bn2_sb, "bn2", cfg, groups)
        d1n = [P_fg.tile([128, Bs], F16, name=f"d1n{hc}", tag=f"d1n{hc}")
               for hc in range(HC)]
        for hc in range(HC):
            nc.scalar.activation(d1n[hc], d1_sb[hc], AF.Relu,
                                 bias=bias2[:, hc:hc + 1],
                                 scale=scale2[:, hc:hc + 1])

        # ---------- Phase G: pred = relu(d1n^T^T @ Wd2) ----------
        if has_bd2:
            bd2_pad = R([128, cfg.D], F16, "bd2_pad")
            nc.vector.memset(bd2_pad, 0.0)
            nc.sync.dma_start(bd2_pad[0:2, :], t["bd2r"])
        with tc.tile_pool(name="w2", bufs=2) as P_w2, \
             tc.tile_pool(name="psG", bufs=2, space="PSUM") as P_psum:
            for dn in range(DN):
                wt = P_w2.tile([128, HC, 512], F16, name="wd2t", tag="wd2t")
                nc.sync.dma_start(wt, t["wd2"][dn])
                pp = [P_psum.tile([128, 512], F32, name=f"pps{bt}", tag=f"pps{bt}")
                      for bt in range(BT)]
                for hc in range(HC):
                    for bt in range(BT):
                        nc.tensor.matmul(
                            pp[bt], d1n[hc][:, bt * 128:(bt + 1) * 128],
                            wt[:, hc], start=(hc == 0),
                            stop=(hc == HC - 1 and not has_bd2))
                if has_bd2:
                    for bt in range(BT):
                        nc.tensor.matmul(
                            pp[bt], ones_pad,
                            bd2_pad[:, dn * 512:(dn + 1) * 512],
                            start=False, stop=True)
                for bt in range(BT):
                    po = P_tmp.tile([128, 512], F32, name="po", tag="po")
                    nc.scalar.activation(po, pp[bt], AF.Relu)
                    nc.sync.dma_start(
                        t["pred_o"][bt * 128:(bt + 1) * 128,
                                    dn * 512:(dn + 1) * 512], po)
        ctx_fg.close()


def _bn_math(nc, P_tmp, sums, sumsq, gamma, beta, scale_out, bias_out, name, B, W):
    """scale/bias for BN from all-reduced sums: scale = g*rsqrt(var+eps),
    bias = beta - mean*scale. All APs are [128, W] slices."""
    AF = mybir.ActivationFunctionType
    OP = mybir.AluOpType

    def pt(nm):
        return P_tmp.tile([128, W], F32, name=f"{name}_{nm}", tag=f"{name}_{nm}")

    mean = pt("mean")
    nc.vector.tensor_scalar(mean, sums, 1.0 / B, None, OP.mult)
    var = pt("var")
    nc.vector.tensor_scalar(var, sumsq, 1.0 / B, None, OP.mult)
    m2 = pt("m2")
    nc.vector.tensor_tensor(m2, mean, mean, OP.mult)
    nc.vector.tensor_tensor(var, var, m2, OP.subtract)
    nc.vector.tensor_scalar(var, var, BN_EPS, None, OP.add)
    inv = pt("inv")
    nc.vector.reciprocal(inv, var)
    rs = pt("rs")
    nc.scalar.activation(rs, inv, AF.Sqrt)
    nc.vector.tensor_tensor(scale_out, rs, gamma, OP.mult)
    nc.vector.tensor_tensor(bias_out, mean, scale_out, OP.mult)
    nc.vector.tensor_tensor(bias_out, beta, bias_out, OP.subtract)


def _bn_allreduce(tc, nc, P_res, P_dram, P_tmp, stat, bnv, name, cfg, groups):
    """All-reduce [128, 2*HC] sums across cores; return per-chunk scale/bias."""
    AF = mybir.ActivationFunctionType
    OP = mybir.AluOpType
    HC, B = cfg.HC, cfg.B
    sin = P_dram.tile([128, 2 * HC], F32, name=f"{name}_in", tag=f"{name}_in")
    sout = P_dram.tile([128, 2 * HC], F32, name=f"{name}_out", tag=f"{name}_out")
    nc.sync.dma_start(sin, stat)
    nc.gpsimd.collective_compute(
        "AllReduce", OP.add, replica_groups=groups,
        ins=[sin.opt()], outs=[sout.opt()])
    tot = P_res.tile([128, 2 * HC], F32, name=f"{name}_tot", tag=f"{name}_tot")
    nc.sync.dma_start(tot, sout)

    def pt(nm):
        return P_tmp.tile([128, HC], F32, name=f"{name}_{nm}", tag=f"{name}_{nm}")

    mean = P_res.tile([128, HC], F32, name=f"{name}_mean", tag=f"{name}_mean")
    nc.vector.tensor_scalar(mean, tot[:, :HC], 1.0 / B, None, OP.mult)
    var = pt("var")
    nc.vector.tensor_scalar(var, tot[:, HC:], 1.0 / B, None, OP.mult)
    m2 = pt("m2")
    nc.vector.tensor_tensor(m2, mean, mean, OP.mult)
    nc.vector.tensor_tensor(var, var, m2, OP.subtract)
    nc.vector.tensor_scalar(var, var, BN_EPS, None, OP.add)
    inv = pt("inv")
    nc.vector.reciprocal(inv, var)
    rs = pt("rs")
    nc.scalar.activation(rs, inv, AF.Sqrt)
    scale = P_res.tile([128, HC], F32, name=f"{name}_scale", tag=f"{name}_scale")
    nc.vector.tensor_tensor(scale, rs, bnv[:, 0], OP.mult)
    bias = P_res.tile([128, HC], F32, name=f"{name}_bias", tag=f"{name}_bias")
    nc.vector.tensor_tensor(bias, mean, scale, OP.mult)
    nc.vector.tensor_tensor(bias, bnv[:, 1], bias, OP.subtract)
    return scale, bias


# ============================ host side ============================

def _split16(a):
    hi = a.astype(np.float16)
    lo = (a.astype(np.float32) - hi.astype(np.float32)).astype(np.float16)
    return hi, lo


def prep_inputs(cfg: Cfg, inputs):
    """Host-side prep: transpose/swizzle/split. Returns per-core in_maps."""
    x = np.asarray(inputs["inputs"], np.float32)
    We1 = np.asarray(inputs["We1"], np.float32)
    We2 = np.asarray(inputs["We2"], np.float32)
    C = np.asarray(inputs["context"], np.float32)
    Wd1 = np.asarray(inputs["Wd1"], np.float32)
    Wd2 = np.asarray(inputs["Wd2"], np.float32)
    ge, bbe = np.asarray(inputs["ge"], np.float32), np.asarray(inputs["bbe"], np.float32)
    gd, bbd = np.asarray(inputs["gd"], np.float32), np.asarray(inputs["bbd"], np.float32)
    be2 = np.asarray(inputs["be2"], np.float32)
    bd2 = np.asarray(inputs["bd2"], np.float32)
    D, H, dm, K = cfg.D, cfg.H, cfg.dm, cfg.K
    DC, HC, KC, DN, Bs = cfg.DC, cfg.HC, cfg.KC, cfg.DN, cfg.Bs

    w1h, w1l = _split16(We1)
    # [D, H] -> [DC, 128(d-part), HC, 128(h-free)] (pure reshape)
    sw1 = lambda w: np.ascontiguousarray(w.reshape(DC, 128, HC, 128))
    we1_hi, we1_lo = sw1(w1h), sw1(w1l)
    w2h, w2l = _split16(We2)
    sw2 = lambda w: np.ascontiguousarray(w.reshape(HC, 128, dm).transpose(1, 0, 2))
    we2_hi, we2_lo = sw2(w2h), sw2(w2l)
    ct2 = np.ascontiguousarray(2.0 * C.T)
    ct2_hi, ct2_lo = _split16(ct2)
    csq = (C.astype(np.float64) ** 2).sum(1)
    csqn = np.stack(_split16((-csq).astype(np.float32)))       # [2, K]
    c_rows = np.ascontiguousarray(C.astype(np.float16))
    wd1 = Wd1.astype(np.float16)                                # [dm, H]
    # [H, D] -> [DN, 128(h-part), HC, 512(d-free)]
    wd2 = np.ascontiguousarray(
        Wd2.astype(np.float16).reshape(HC, 128, DN, 512).transpose(2, 1, 0, 3))
    stripe = lambda v: np.ascontiguousarray(v.reshape(HC, 128).T)
    bn1v = np.ascontiguousarray(np.stack([stripe(ge), stripe(bbe)], axis=1))
    bn2v = np.ascontiguousarray(np.stack([stripe(gd), stripe(bbd)], axis=1))
    be2c = np.ascontiguousarray(be2.reshape(dm, 1))
    has_bd2 = bool(np.any(bd2 != 0.0))

    shared = dict(
        we1_hi=we1_hi, we1_lo=we1_lo, we2_hi=we2_hi, we2_lo=we2_lo,
        ct2_hi=ct2_hi, ct2_lo=ct2_lo, csqn=csqn, c_rows=c_rows,
        wd1=wd1, wd2=wd2, bn1v=bn1v, bn2v=bn2v, be2c=be2c)
    if has_bd2:
        shared["bd2r"] = np.stack(_split16(bd2))

    xT = np.ascontiguousarray(x.T)                              # [D, B]
    in_maps = []
    for i in range(cfg.n_cores):
        xs = xT[:, i * Bs:(i + 1) * Bs]
        xh, xl = _split16(xs)
        swx = lambda a: np.ascontiguousarray(a.reshape(DC, 128, Bs))
        m = dict(shared)
        m["xt_hi"], m["xt_lo"] = swx(xh), swx(xl)
        in_maps.append(m)
    return in_maps, has_bd2


_GRAPH_CACHE = {}


def run(cfg: Cfg, inputs, trace=False, trace_kwargs=None):
    in_maps, has_bd2 = prep_inputs(cfg, inputs)
    key = (cfg.B, cfg.D, cfg.H, cfg.dm, cfg.K, cfg.n_cores, has_bd2)
    if key not in _GRAPH_CACHE:
        _GRAPH_CACHE[key] = build_graph(cfg, has_bd2)
    nc = _GRAPH_CACHE[key]
    res = run_bass_kernel_spmd(
        nc, in_maps, core_ids=list(range(cfg.n_cores)),
        trace=trace, **(trace_kwargs or {}))
    pred = np.concatenate([res.results[i]["pred"] for i in range(cfg.n_cores)])
    assign = np.concatenate([res.results[i]["assign"] for i in range(cfg.n_cores)])
    feat = np.concatenate([res.results[i]["feat"] for i in range(cfg.n_cores)])
    return (pred, assign, feat), res


def kernel(**inputs):
    (pred, assign, feat), _ = run(FULL_CFG, inputs)
    return pred, assign, feat


# revision 25
# speedup vs baseline: 1.0387x; 1.0387x over previous
"""Trainium2 Bass kernel for nn_AEModel (VQ autoencoder), 8-core data-parallel.

Architecture: Dense(D->H) -> BN(train) -> ReLU -> Dense(H->dm) -> VQ nearest
codebook (K codes) -> Dense(dm->H) -> BN -> ReLU -> Dense(H->D) -> ReLU.
Returns (pred [B,D], assignment=-dist [B,K], feat [B,dm]).

Sharding: data-parallel over batch (B/8 rows per core); weights replicated;
BN statistics all-reduced (2 x 16KB collectives).

Precision: encoder + VQ distance use fp16 hi/lo split matmuls (3 matmuls per
fp32 matmul, ~22-bit effective mantissa) so the argmin matches the CPU fp32
reference exactly; decoder uses plain fp16 (tolerance is ample post-VQ).
Quantization is one-hot: onehot = (assign == rowmax) compared against values
from the same tensor, then quant = C^T @ onehot^T via PE transposes.
"""

import sys

try:
    import concourse.bass as bass  # noqa: F401
except ImportError:
    sys.path.insert(0, "/opt/trn_rl_repo")

import numpy as np

import concourse.bass as bass  # noqa: F811
import concourse.mybir as mybir
import concourse.tile as tile
from concourse import bacc
from concourse.bass_utils import run_bass_kernel_spmd
from concourse.masks import make_identity

F32 = mybir.dt.float32
F16 = mybir.dt.float16

BN_EPS = 1e-3


class Cfg:
    def __init__(self, B=4096, D=4096, H=2048, dm=128, K=4096, n_cores=8):
        self.B, self.D, self.H, self.dm, self.K, self.n_cores = B, D, H, dm, K, n_cores
        self.Bs = B // n_cores          # batch rows per core
        self.BT = self.Bs // 128        # batch row-tiles per core
        self.DC = D // 128              # input-dim chunks
        self.HC = H // 128              # hidden chunks
        self.KC = K // 128              # code chunks (partition-sized)
        self.KN = K // 512              # code chunks (free-dim-sized)
        self.DN = D // 512              # output-dim chunks
        assert self.Bs % 128 == 0 and D % 128 == 0 and H % 128 == 0
        assert dm == 128 and K % 512 == 0 and D % 512 == 0


FULL_CFG = Cfg()


def build_graph(cfg: Cfg, has_bd2: bool = False):
    """Build the SPMD graph (same graph runs on all cores)."""
    nc = bacc.Bacc(
        "TRN2",
        target_bir_lowering=False,
        debug=False,
        num_devices=cfg.n_cores,
    )
    Bs, DC, HC, KC, DN = cfg.Bs, cfg.DC, cfg.HC, cfg.KC, cfg.DN
    K, dm = cfg.K, cfg.dm

    di = lambda n, s, d=F16: nc.dram_tensor(n, s, d, kind="ExternalInput").ap()
    t = {}
    t["xt_hi"] = di("xt_hi", [DC, 128, Bs])      # x^T shard, split, dc-major
    t["xt_lo"] = di("xt_lo", [DC, 128, Bs])
    t["we1_hi"] = di("we1_hi", [DC, 128, HC, 128])
    t["we1_lo"] = di("we1_lo", [DC, 128, HC, 128])
    t["we2_hi"] = di("we2_hi", [128, HC, dm])    # [h-part, hc, dm]
    t["we2_lo"] = di("we2_lo", [128, HC, dm])
    t["ct2_hi"] = di("ct2_hi", [dm, K])          # 2*C^T, split
    t["ct2_lo"] = di("ct2_lo", [dm, K])
    t["csqn"] = di("csqn", [2, K])               # rows: hi, lo of (-|c|^2)
    t["c_rows"] = di("c_rows", [K, dm])          # C fp16 row table (gather)
    t["wd1"] = di("wd1", [dm, cfg.H])
    t["wd2"] = di("wd2", [DN, 128, HC, 512])     # [dn, h-part, hc, d-free]
    t["bn1v"] = di("bn1v", [128, 2, HC], F32)    # [:,0]=ge striped, [:,1]=bbe
    t["bn2v"] = di("bn2v", [128, 2, HC], F32)
    t["be2c"] = di("be2c", [dm, 1], F32)
    if has_bd2:
        t["bd2r"] = di("bd2r", [2, cfg.D])

    do = lambda n, s: nc.dram_tensor(n, s, F32, kind="ExternalOutput").ap()
    t["pred_o"] = do("pred", [Bs, cfg.D])
    t["assign_o"] = do("assign", [Bs, K])
    t["feat_o"] = do("feat", [Bs, dm])

    with tile.TileContext(nc) as tc:
        _body(tc, nc, cfg, t, has_bd2)
    nc.compile()
    return nc


def _body(tc, nc, cfg, t, has_bd2):
    Bs, BT, DC, HC, KC, KN, DN = (
        cfg.Bs, cfg.BT, cfg.DC, cfg.HC, cfg.KC, cfg.KN, cfg.DN)
    K, dm, B, H = cfg.K, cfg.dm, cfg.B, cfg.H
    groups = [list(range(cfg.n_cores))]
    AF = mybir.ActivationFunctionType
    OP = mybir.AluOpType
    AX = mybir.AxisListType

    from contextlib import ExitStack
    ctx = ExitStack()
    with ctx:
        P_res = ctx.enter_context(tc.tile_pool(name="res", bufs=1))
        P_dram = ctx.enter_context(tc.tile_pool(name="drp", bufs=1, space="DRAM"))
        P_tmp = ctx.enter_context(tc.tile_pool(name="tmp", bufs=3))

        def R(shape, dtype, name):       # resident tile, own slot
            return P_res.tile(shape, dtype, name=name, tag=name)

        # ---------- resident loads ----------
        ident16 = R([128, 128], F16, "ident16")
        make_identity(nc, ident16)
        ident32 = R([128, 128], F32, "ident32")
        make_identity(nc, ident32)

        def load(name, shape, dtype=F16):
            sb = R(shape, dtype, name + "_sb")
            nc.sync.dma_start(sb, t[name])
            return sb


        stat1 = R([128, HC, 2], F32, "stat1")    # [:, hc, 0]=sum, [:, hc, 1]=sumsq
        tot1 = R([128, HC, 2], F32, "tot1")
        stat2 = R([128, 2 * HC], F32, "stat2")
        fsq = R([128, BT], F32, "fsq")
        bn1_sb = load("bn1v", [128, 2, HC], F32)

        # ---------- Phase A: h^T = We1^T @ x^T (fp16 split x3) ----------
        # dc-outer / hc-inner over groups of GH psum banks; x^T and We1 both
        # streamed (x^T is re-read once per group). BN1 statistics are
        # all-reduced per group so the collective overlaps the next group's
        # matmuls.
        ctx_abc = ExitStack()
        P_h = ctx_abc.enter_context(tc.tile_pool(name="hsb", bufs=1))
        h_sb = [P_h.tile([128, Bs], F32, name=f"h{hc}", tag=f"h{hc}")
                for hc in range(HC)]
        GH = min(8, HC)
        NGRP = HC // GH
        with tc.tile_pool(name="xts", bufs=3) as P_xts, \
             tc.tile_pool(name="w1", bufs=3) as P_w1, \
             tc.tile_pool(name="psA", bufs=1, space="PSUM") as P_psA:
            for g in range(NGRP):
                pss = [P_psA.tile([128, Bs], F32, name=f"hps{j}", tag=f"hps{j}")
                       for j in range(GH)]
                for dc in range(DC):
                    xh = P_xts.tile([128, Bs], F16, name="xsh", tag="xsh")
                    nc.sync.dma_start(xh, t["xt_hi"][dc])
                    xl = P_xts.tile([128, Bs], F16, name="xsl", tag="xsl")
                    nc.sync.dma_start(xl, t["xt_lo"][dc])
                    gs = slice(g * GH, (g + 1) * GH)
                    wh = P_w1.tile([128, GH, 128], F16, name="w1h", tag="w1h")
                    nc.sync.dma_start(wh, t["we1_hi"][dc, :, gs, :])
                    wl = P_w1.tile([128, GH, 128], F16, name="w1l", tag="w1l")
                    nc.sync.dma_start(wl, t["we1_lo"][dc, :, gs, :])
                    for j in range(GH):
                        ps = pss[j]
                        nc.tensor.matmul(ps, wh[:, j], xh,
                                         start=(dc == 0), stop=False)
                        nc.tensor.matmul(ps, wh[:, j], xl,
                                         start=False, stop=False)
                        nc.tensor.matmul(ps, wl[:, j], xh,
                                         start=False, stop=(dc == DC - 1))
                for j in range(GH):
                    hc = g * GH + j
                    nc.scalar.activation(h_sb[hc], pss[j], AF.Identity,
                                         accum_out=stat1[:, hc, 0:1])
                    sq = P_tmp.tile([128, Bs], F32, name="sq", tag="sq")
                    nc.scalar.activation(sq, h_sb[hc], AF.Square,
                                         accum_out=stat1[:, hc, 1:2])
                # group all-reduce (overlaps the next group's matmuls)
                gs = slice(g * GH, (g + 1) * GH)
                sin = P_dram.tile([128, GH, 2], F32, name=f"b1i{g}", tag=f"b1i{g}")
                sout = P_dram.tile([128, GH, 2], F32, name=f"b1o{g}", tag=f"b1o{g}")
                nc.sync.dma_start(sin, stat1[:, gs, :])
                nc.gpsimd.collective_compute(
                    "AllReduce", OP.add, replica_groups=groups,
                    ins=[sin.opt()], outs=[sout.opt()])
                nc.sync.dma_start(tot1[:, gs, :], sout)

        if True:
            # ---------- Phase B/C: BN apply + feat^T, per group ----------
            we2_hi_sb = load("we2_hi", [128, HC, dm])
            we2_lo_sb = load("we2_lo", [128, HC, dm])
            be2_sb = load("be2c", [dm, 1], F32)
            scale1 = R([128, HC], F32, "scale1")
            bias1 = R([128, HC], F32, "bias1")
            P_psum = ctx_abc.enter_context(
                tc.tile_pool(name="psBC", bufs=2, space="PSUM"))
            hn_hi = [P_h.tile([128, Bs], F16, name=f"hnh{hc}", tag=f"hnh{hc}")
                     for hc in range(HC)]
            hn_lo = [P_h.tile([128, Bs], F16, name=f"hnl{hc}", tag=f"hnl{hc}")
                     for hc in range(HC)]
            fps = P_psum.tile([dm, Bs], F32, name="fps", tag="fps", bufs=1)
            for g in range(NGRP):
                gs = slice(g * GH, (g + 1) * GH)
                _bn_math(nc, P_tmp, tot1[:, gs, 0], tot1[:, gs, 1],
                         bn1_sb[:, 0, gs], bn1_sb[:, 1, gs],
                         scale1[:, gs], bias1[:, gs], f"bn1g{g}", B, GH)
                for j in range(GH):
                    hc = g * GH + j
                    hn = P_tmp.tile([128, Bs], F32, name="hn", tag="hn")
                    nc.scalar.activation(hn, h_sb[hc], AF.Relu,
                                         bias=bias1[:, hc:hc + 1],
                                         scale=scale1[:, hc:hc + 1])
                    nc.vector.tensor_copy(hn_hi[hc], hn)
                    nc.vector.tensor_tensor(hn_lo[hc], hn, hn_hi[hc], OP.subtract)
                    nc.tensor.matmul(fps, we2_hi_sb[:, hc], hn_hi[hc],
                                     start=(hc == 0), stop=False)
                    nc.tensor.matmul(fps, we2_hi_sb[:, hc], hn_lo[hc],
                                     start=False, stop=False)
                    nc.tensor.matmul(fps, we2_lo_sb[:, hc], hn_hi[hc],
                                     start=False, stop=(hc == HC - 1))

            featT = R([dm, Bs], F32, "featT")
            nc.scalar.activation(featT, fps, AF.Identity, bias=be2_sb[:, 0:1])
            fT_hi = R([dm, Bs], F16, "fT_hi")
            nc.vector.tensor_copy(fT_hi, featT)
            fT_lo = R([dm, Bs], F16, "fT_lo")
            nc.vector.tensor_tensor(fT_lo, featT, fT_hi, OP.subtract)

            # feat output ([Bs, dm]) + per-row |f|^2 via PE transpose
            for bt in range(BT):
                tp = P_psum.tile([128, 128], F32, name="tps32", tag="tps32")
                nc.tensor.transpose(tp, featT[:, bt * 128:(bt + 1) * 128], ident32)
                fb = P_tmp.tile([128, dm], F32, name="fbk", tag="fbk")
                nc.scalar.activation(fb, tp, AF.Identity)
                nc.sync.dma_start(t["feat_o"][bt * 128:(bt + 1) * 128, :], fb)
                sqf = P_tmp.tile([128, dm], F32, name="sqf", tag="sqf")
                nc.scalar.activation(sqf, fb, AF.Square,
                                     accum_out=fsq[:, bt:bt + 1])

        ctx_abc.close()

        # ---------- Phase D+E: assignment/-dist, onehot, quant^T ----------
        ct2_hi_sb = load("ct2_hi", [dm, K])
        ct2_lo_sb = load("ct2_lo", [dm, K])
        # -|c|^2 as rows 0..1 of a zero-padded [128, K] tile so the csq add is
        # a full-K matmul (K=2 matmuls measured 2.2x slower than K=128).
        csqn_pad = R([128, K], F16, "csqn_pad")
        nc.vector.memset(csqn_pad, 0.0)
        nc.sync.dma_start(csqn_pad[0:2, :], t["csqn"])
        ones_pad = R([128, 128], F16, "ones_pad")
        nc.vector.memset(ones_pad, 0.0)
        nc.vector.memset(ones_pad[0:2, :], 1.0)

        quant16 = R([dm, Bs], F16, "quant16")
        wd1_sb = load("wd1", [dm, H])
        bn2_sb = load("bn2v", [128, 2, HC], F32)
        stat2p = R([128, 2, HC, BT], F32, "stat2p")
        ctx_fg = ExitStack()
        P_fg = ctx_fg.enter_context(tc.tile_pool(name="fg", bufs=1))
        d1_sb = [P_fg.tile([128, Bs], F32, name=f"d1{hc}", tag=f"d1{hc}")
                 for hc in range(HC)]
        with tc.tile_pool(name="asn", bufs=2) as P_asn, \
             tc.tile_pool(name="psD", bufs=2, space="PSUM") as P_psum:
            for bt in range(BT):
                asn = P_asn.tile([128, K], F32, name="asn", tag="asn")
                fh = fT_hi[:, bt * 128:(bt + 1) * 128]
                fl = fT_lo[:, bt * 128:(bt + 1) * 128]
                for kn in range(KN):
                    ks = slice(kn * 512, (kn + 1) * 512)
                    dp = P_psum.tile([128, 512], F32, name="dps", tag="dps")
                    nc.tensor.matmul(dp, fh, ct2_hi_sb[:, ks], start=True, stop=False)
                    nc.tensor.matmul(dp, fh, ct2_lo_sb[:, ks], start=False, stop=False)
                    nc.tensor.matmul(dp, fl, ct2_hi_sb[:, ks], start=False, stop=False)
                    nc.tensor.matmul(dp, ones_pad, csqn_pad[:, ks],
                                     start=False, stop=True)
                    nc.vector.tensor_scalar(
                        asn[:, ks], dp, fsq[:, bt:bt + 1], None, OP.subtract)
                nc.sync.dma_start(t["assign_o"][bt * 128:(bt + 1) * 128, :], asn)
                # hardware argmax (top-8 values + indices), then gather the
                # winning codebook rows and transpose into quant^T columns
                mx8 = P_tmp.tile([128, 8], F32, name="mx8", tag="mx8")
                nc.vector.max(out=mx8, in_=asn)
                idx8 = P_tmp.tile([128, 8], mybir.dt.uint32, name="idx8", tag="idx8")
                nc.vector.max_index(idx8, mx8, asn)
                qbk = P_tmp.tile([128, dm], F16, name="qbk", tag="qbk")
                nc.gpsimd.indirect_dma_start(
                    out=qbk, out_offset=None, in_=t["c_rows"],
                    in_offset=bass.IndirectOffsetOnAxis(ap=idx8[:, 0:1], axis=0))
                tq = P_psum.tile([128, dm], F16, name="tq", tag="tq")
                nc.tensor.transpose(tq, qbk, ident16)
                bs_ = slice(bt * 128, (bt + 1) * 128)
                nc.scalar.activation(quant16[:, bs_], tq, AF.Identity)
                # d1^T column block for this bt + its BN2 stat contributions
                for hcg in range(HC // 4):
                    d1p = P_psum.tile([128, 4, 128], F32, name="d1ps", tag="d1ps")
                    for j in range(4):
                        hc = hcg * 4 + j
                        nc.tensor.matmul(
                            d1p[:, j], wd1_sb[:, hc * 128:(hc + 1) * 128],
                            quant16[:, bs_], start=True, stop=True)
                    for j in range(4):
                        hc = hcg * 4 + j
                        nc.scalar.activation(
                            d1_sb[hc][:, bs_], d1p[:, j], AF.Identity,
                            accum_out=stat2p[:, 0, hc, bt:bt + 1])
                        sq = P_tmp.tile([128, 128], F32, name="sqd", tag="sqd")
                        nc.scalar.activation(
                            sq, d1_sb[hc][:, bs_], AF.Square,
                            accum_out=stat2p[:, 1, hc, bt:bt + 1])

        # ---------- Phase F: BN2 all-reduce + apply ----------
        nc.vector.tensor_reduce(stat2, stat2p, axis=AX.X, op=OP.add)
        scale2, bias2 = _bn_allreduce(
            tc, nc, P_res, P_dram, P_tmp, stat2, bn2_sb, "bn2", cfg, groups)
        d1n = [P_fg.tile([128, Bs], F16, name=f"d1n{hc}", tag=f"d1n{hc}")
               for hc in range(HC)]
        for hc in range(HC):
            nc.scalar.activation(d1n[hc], d1_sb[hc], AF.Relu,
                                 bias=bias2[:, hc:hc + 1],
                                 scale=scale2[:, hc:hc + 1])

        # ---------- Phase G: pred = relu(d1n^T^T @ Wd2) ----------
        if has_bd2:
            bd2_pad = R([128, cfg.D], F16, "bd2_pad")
            nc.vector.memset(bd2_pad, 0.0)
            nc.sync.dma_start(bd2_pad[0:2, :], t["bd2r"])
        with tc.tile_pool(name="w2", bufs=2) as P_w2, \
             tc.tile_pool(name="psG", bufs=2, space="PSUM") as P_psum:
            for dn in range(DN):
                wt = P_w2.tile([128, HC, 512], F16, name="wd2t", tag="wd2t")
                nc.sync.dma_start(wt, t["wd2"][dn])
                pp = [P_psum.tile([128, 512], F32, name=f"pps{bt}", tag=f"pps{bt}")
                      for bt in range(BT)]
                for hc in range(HC):
                    for bt in range(BT):
                        nc.tensor.matmul(
                            pp[bt], d1n[hc][:, bt * 128:(bt + 1) * 128],
                            wt[:, hc], start=(hc == 0),
                            stop=(hc == HC - 1 and not has_bd2))
                if has_bd2:
                    for bt in range(BT):
                        nc.tensor.matmul(
                            pp[bt], ones_pad,
                            bd2_pad[:, dn * 512:(dn + 1) * 512],
                            start=False, stop=True)
                for bt in range(BT):
                    po = P_tmp.tile([128, 512], F32, name="po", tag="po")
                    nc.scalar.activation(po, pp[bt], AF.Relu)
                    nc.sync.dma_start(
                        t["pred_o"][bt * 128:(bt + 1) * 128,
                                    dn * 512:(dn + 1) * 512], po)
        ctx_fg.close()


def _bn_math(nc, P_tmp, sums, sumsq, gamma, beta, scale_out, bias_out, name, B, W):
    """scale/bias for BN from all-reduced sums: scale = g*rsqrt(var+eps),
    bias = beta - mean*scale. All APs are [128, W] slices."""
    AF = mybir.ActivationFunctionType
    OP = mybir.AluOpType

    def pt(nm):
        return P_tmp.tile([128, W], F32, name=f"{name}_{nm}", tag=f"{name}_{nm}")

    mean = pt("mean")
    nc.vector.tensor_scalar(mean, sums, 1.0 / B, None, OP.mult)
    var = pt("var")
    nc.vector.tensor_scalar(var, sumsq, 1.0 / B, None, OP.mult)
    m2 = pt("m2")
    nc.vector.tensor_tensor(m2, mean, mean, OP.mult)
    nc.vector.tensor_tensor(var, var, m2, OP.subtract)
    nc.vector.tensor_scalar(var, var, BN_EPS, None, OP.add)
    inv = pt("inv")
    nc.vector.reciprocal(inv, var)
    rs = pt("rs")
    nc.scalar.activation(rs, inv, AF.Sqrt)
    nc.vector.tensor_tensor(scale_out, rs, gamma, OP.mult)
    nc.vector.tensor_tensor(bias_out, mean, scale_out, OP.mult)
    nc.vector.tensor_tensor(bias_out, beta, bias_out, OP.subtract)


def _bn_allreduce(tc, nc, P_res, P_dram, P_tmp, stat, bnv, name, cfg, groups):
    """All-reduce [128, 2*HC] sums across cores; return per-chunk scale/bias."""
    AF = mybir.ActivationFunctionType
    OP = mybir.AluOpType
    HC, B = cfg.HC, cfg.B
    sin = P_dram.tile([128, 2 * HC], F32, name=f"{name}_in", tag=f"{name}_in")
    sout = P_dram.tile([128, 2 * HC], F32, name=f"{name}_out", tag=f"{name}_out")
    nc.sync.dma_start(sin, stat)
    nc.gpsimd.collective_compute(
        "AllReduce", OP.add, replica_groups=groups,
        ins=[sin.opt()], outs=[sout.opt()])
    tot = P_res.tile([128, 2 * HC], F32, name=f"{name}_tot", tag=f"{name}_tot")
    nc.sync.dma_start(tot, sout)

    def pt(nm):
        return P_tmp.tile([128, HC], F32, name=f"{name}_{nm}", tag=f"{name}_{nm}")

    mean = P_res.tile([128, HC], F32, name=f"{name}_mean", tag=f"{name}_mean")
    nc.vector.tensor_scalar(mean, tot[:, :HC], 1.0 / B, None, OP.mult)
    var = pt("var")
    nc.vector.tensor_scalar(var, tot[:, HC:], 1.0 / B, None, OP.mult)
    m2 = pt("m2")
    nc.vector.tensor_tensor(m2, mean, mean, OP.mult)
    nc.vector.tensor_tensor(var, var, m2, OP.subtract)
    nc.vector.tensor_scalar(var, var, BN_EPS, None, OP.add)
    inv = pt("inv")
    nc.vector.reciprocal(inv, var)
    rs = pt("rs")
    nc.scalar.activation(rs, inv, AF.Sqrt)
    scale = P_res.tile([128, HC], F32, name=f"{name}_scale", tag=f"{name}_scale")
    nc.vector.tensor_tensor(scale, rs, bnv[:, 0], OP.mult)
    bias = P_res.tile([128, HC], F32, name=f"{name}_bias", tag=f"{name}_bias")
    nc.vector.tensor_tensor(bias, mean, scale, OP.mult)
    nc.vector.tensor_tensor(bias, bnv[:, 1], bias, OP.subtract)
    return scale, bias


# ============================ host side ============================

def _split16(a):
    hi = a.astype(np.float16)
    lo = (a.astype(np.float32) - hi.astype(np.float32)).astype(np.float16)
    return hi, lo


def prep_inputs(cfg: Cfg, inputs):
    """Host-side prep: transpose/swizzle/split. Returns per-core in_maps."""
    x = np.asarray(inputs["inputs"], np.float32)
    We1 = np.asarray(inputs["We1"], np.float32)
    We2 = np.asarray(inputs["We2"], np.float32)
    C = np.asarray(inputs["context"], np.float32)
    Wd1 = np.asarray(inputs["Wd1"], np.float32)
    Wd2 = np.asarray(inputs["Wd2"], np.float32)
    ge, bbe = np.asarray(inputs["ge"], np.float32), np.asarray(inputs["bbe"], np.float32)
    gd, bbd = np.asarray(inputs["gd"], np.float32), np.asarray(inputs["bbd"], np.float32)
    be2 = np.asarray(inputs["be2"], np.float32)
    bd2 = np.asarray(inputs["bd2"], np.float32)
    D, H, dm, K = cfg.D, cfg.H, cfg.dm, cfg.K
    DC, HC, KC, DN, Bs = cfg.DC, cfg.HC, cfg.KC, cfg.DN, cfg.Bs

    w1h, w1l = _split16(We1)
    # [D, H] -> [DC, 128(d-part), HC, 128(h-free)] (pure reshape)
    sw1 = lambda w: np.ascontiguousarray(w.reshape(DC, 128, HC, 128))
    we1_hi, we1_lo = sw1(w1h), sw1(w1l)
    w2h, w2l = _split16(We2)
    sw2 = lambda w: np.ascontiguousarray(w.reshape(HC, 128, dm).transpose(1, 0, 2))
    we2_hi, we2_lo = sw2(w2h), sw2(w2l)
    ct2 = np.ascontiguousarray(2.0 * C.T)
    ct2_hi, ct2_lo = _split16(ct2)
    csq = (C.astype(np.float64) ** 2).sum(1)
    csqn = np.stack(_split16((-csq).astype(np.float32)))       # [2, K]
    c_rows = np.ascontiguousarray(C.astype(np.float16))
    wd1 = Wd1.astype(np.float16)                                # [dm, H]
    # [H, D] -> [DN, 128(h-part), HC, 512(d-free)]
    wd2 = np.ascontiguousarray(
        Wd2.astype(np.float16).reshape(HC, 128, DN, 512).transpose(2, 1, 0, 3))
    stripe = lambda v: np.ascontiguousarray(v.reshape(HC, 128).T)
    bn1v = np.ascontiguousarray(np.stack([stripe(ge), stripe(bbe)], axis=1))
    bn2v = np.ascontiguousarray(np.stack([stripe(gd), stripe(bbd)], axis=1))
    be2c = np.ascontiguousarray(be2.reshape(dm, 1))
    has_bd2 = bool(np.any(bd2 != 0.0))

    shared = dict(
        we1_hi=we1_hi, we1_lo=we1_lo, we2_hi=we2_hi, we2_lo=we2_lo,
        ct2_hi=ct2_hi, ct2_lo=ct2_lo, csqn=csqn, c_rows=c_rows,
        wd1=wd1, wd2=wd2, bn1v=bn1v, bn2v=bn2v, be2c=be2c)
    if has_bd2:
        shared["bd2r"] = np.stack(_split16(bd2))

    xT = np.ascontiguousarray(x.T)                              # [D, B]
    in_maps = []
    for i in range(cfg.n_cores):
        xs = xT[:, i * Bs:(i + 1) * Bs]
        xh, xl = _split16(xs)
        swx = lambda a: np.ascontiguousarray(a.reshape(DC, 128, Bs))
        m = dict(shared)
        m["xt_hi"], m["xt_lo"] = swx(xh), swx(xl)
        in_maps.append(m)
    return in_maps, has_bd2


_GRAPH_CACHE = {}


def run(cfg: Cfg, inputs, trace=False, trace_kwargs=None):
    in_maps, has_bd2 = prep_inputs(cfg, inputs)
    key = (cfg.B, cfg.D, cfg.H, cfg.dm, cfg.K, cfg.n_cores, has_bd2)
    if key not in _GRAPH_CACHE:
        _GRAPH_CACHE[key] = build_graph(cfg, has_bd2)
    nc = _GRAPH_CACHE[key]
    res = run_bass_kernel_spmd(
        nc, in_maps, core_ids=list(range(cfg.n_cores)),
        trace=trace, **(trace_kwargs or {}))
    pred = np.concatenate([res.results[i]["pred"] for i in range(cfg.n_cores)])
    assign = np.concatenate([res.results[i]["assign"] for i in range(cfg.n_cores)])
    feat = np.concatenate([res.results[i]["feat"] for i in range(cfg.n_cores)])
    return (pred, assign, feat), res


def kernel(**inputs):
    (pred, assign, feat), _ = run(FULL_CFG, inputs)
    return pred, assign, feat
